# revision 2
# baseline (speedup 1.0000x reference)
"""GraphSAGE 2-layer encoder on 8 TRN2 NeuronCores — single-launch version.

Strategy (dst-sharded, fully on-device):
- Nodes sharded 8x12500 by dst range; core k computes output rows for its
  nodes. Inputs shipped over the axon tunnel are minimal: the core's x shard
  (bf16, canonical-padded [12544,128]), a compact int32 edge schedule
  (slot->src canonical index + slot->dst-in-cell), 1/deg, and the weights.
- On device: AllGather assembles the full bf16 feature table in DRAM
  ([8*12544,128]); per 128-slot tile an indirect DMA gathers the edge
  messages; segment-sum runs on the PE as one-hot matmuls (S built by
  iota-compare on DVE) accumulating feature-major aggregates in PSUM; DVE
  scales by 1/deg; stationary W_l/W_r matmuls transform; ScalarE fuses
  bias+ReLU. Layer 1 additionally PE-transposes its output back to row-major
  bf16 and AllGathers it as the layer-2 message table. Both layers run in ONE
  compiled program / ONE launch; only the transposed bf16 output shard
  [128,12544] returns to the host.
"""
import os
import numpy as np
import ml_dtypes

import concourse.bass as bass
import concourse.tile as tile
from concourse import bacc, mybir
from concourse.bass_utils import run_bass_kernel_spmd
from concourse.masks import make_identity

N_NODES = 100000
N_CORES = 8
OWN = N_NODES // N_CORES          # 12500
D = 128
CELL = 128                        # node-columns per cell (= S width = MM N)
N_CELLS = (OWN + CELL - 1) // CELL  # 98
N_CANON = N_CELLS * CELL          # 12544
CANON_ALL = N_CORES * N_CANON     # 100352
BANK_CELLS = 4                    # cells per PSUM bank (512 cols)
N_BANKS = (N_CELLS + BANK_CELLS - 1) // BANK_CELLS  # 25

BF16 = mybir.dt.bfloat16
F32 = mybir.dt.float32
F32R = mybir.dt.float32r
I32 = mybir.dt.int32

_cache = {}


def _build_program(T_cells):
    """Both layers in one SPMD program. T_cells[c] = #128-slot tiles for cell c."""
    TOT_T = int(np.sum(T_cells))
    nc = bacc.Bacc()

    xsh_d = nc.declare_dram_parameter("xsh", [N_CANON, D], BF16, isOutput=False)
    sidx_d = nc.declare_dram_parameter("sidx", [128, max(TOT_T, 1)], I32, isOutput=False)
    dstc_d = nc.declare_dram_parameter("dstc", [128, max(TOT_T, 1)], BF16, isOutput=False)
    inv_d = nc.declare_dram_parameter("invc", [1, N_CANON], F32, isOutput=False)
    wl0_d = nc.declare_dram_parameter("wl0", [128, 128], F32R, isOutput=False)
    wr0_d = nc.declare_dram_parameter("wr0", [128, 128], F32R, isOutput=False)
    b0_d = nc.declare_dram_parameter("b0", [128, 1], F32, isOutput=False)
    wl1_d = nc.declare_dram_parameter("wl1", [128, 128], F32R, isOutput=False)
    wr1_d = nc.declare_dram_parameter("wr1", [128, 128], F32R, isOutput=False)
    b1_d = nc.declare_dram_parameter("b1", [128, 1], F32, isOutput=False)
    iota_d = nc.declare_dram_parameter("iota", [1, CELL], BF16, isOutput=False)
    out_d = nc.declare_dram_parameter("outT", [128, N_CANON], BF16, isOutput=True)

    # internal DRAM: collective bounce/result buffers + layer-1 transposed output
    xin_d = nc.dram_tensor("xin", [N_CANON, D], BF16)
    xfull_d = nc.dram_tensor("xfull", [CANON_ALL, D], BF16, addr_space="Shared")
    h1sh_d = nc.dram_tensor("h1sh", [N_CANON, D], BF16)
    h1full_d = nc.dram_tensor("h1full", [CANON_ALL, D], BF16, addr_space="Shared")
    h1T_d = nc.dram_tensor("h1T", [128, N_CANON], F32R)

    # bank plan: (cell_start, n_cells, tile ranges)
    banks = []
    t0 = 0
    for bk in range(N_BANKS):
        c0 = bk * BANK_CELLS
        ncell = min(BANK_CELLS, N_CELLS - c0)
        tiles = []  # (t_global, cell_off_in_bank)
        for ci in range(ncell):
            for _ in range(T_cells[c0 + ci]):
                tiles.append((t0, ci))
                t0 += 1
        banks.append((c0, ncell, tiles))

    T_BANK_MAX = max(max(len(b[2]) for b in banks), 1)
    GROUPS = [[k for k in range(N_CORES)]]

    with tile.TileContext(nc) as tc:
        with (
            tc.tile_pool(name="singles", bufs=1) as singles,
            tc.tile_pool(name="msgp", bufs=3) as msgp,
            tc.tile_pool(name="sp", bufs=3) as sp,
            tc.tile_pool(name="ownp", bufs=2) as ownp,
            tc.tile_pool(name="mp", bufs=2) as mp,
            tc.tile_pool(name="outp", bufs=3) as outp,
            tc.tile_pool(name="stp", bufs=4) as stp,
            tc.tile_pool(name="psa", bufs=3, space="PSUM") as psa,
            tc.tile_pool(name="pst", bufs=2, space="PSUM") as pst,
            tc.tile_pool(name="ptr", bufs=1, space="PSUM") as ptr,
        ):
            # ---- constants ----
            sidx_t = singles.tile([128, max(TOT_T, 1)], I32)
            nc.sync.dma_start(out=sidx_t[:], in_=sidx_d[:])
            dstc_t = singles.tile([128, max(TOT_T, 1)], BF16)
            nc.sync.dma_start(out=dstc_t[:], in_=dstc_d[:])
            iota_t = singles.tile([128, CELL], BF16)
            nc.gpsimd.dma_start(
                out=iota_t[:],
                in_=bass.AP(tensor=iota_d[:].tensor, offset=0, ap=[[0, 128], [1, CELL]]),
            )
            inv_t = singles.tile([128, N_CANON], F32)
            nc.gpsimd.dma_start(
                out=inv_t[:],
                in_=bass.AP(tensor=inv_d[:].tensor, offset=0, ap=[[0, 128], [1, N_CANON]]),
            )
            w_t = {}
            for nm, hd in (("wl0", wl0_d), ("wr0", wr0_d), ("wl1", wl1_d), ("wr1", wr1_d)):
                w_t[nm] = singles.tile([128, 128], F32R, name=f"w_{nm}")
                nc.sync.dma_start(out=w_t[nm][:], in_=hd[:])
            b_t = {}
            for nm, hd in (("b0", b0_d), ("b1", b1_d)):
                b_t[nm] = singles.tile([128, 1], F32, name=f"b_{nm}")
                nc.sync.dma_start(out=b_t[nm][:], in_=hd[:])
            ident_t = singles.tile([128, 128], BF16)
            make_identity(nc, ident_t[:])
            zeros_t = singles.tile([128, 512], BF16)
            nc.vector.memset(zeros_t[:], 0.0)

            # ---- AllGather x shard -> full table ----
            nc.gpsimd.dma_start(out=xin_d[:], in_=xsh_d[:])
            nc.gpsimd.collective_compute(
                "AllGather", mybir.AluOpType.bypass, replica_groups=GROUPS,
                ins=[xin_d[:]], outs=[xfull_d[:]],
            )

            def layer(table_d, wl, wr, bb, is_last):
                for bk, (c0, ncell, tiles) in enumerate(banks):
                    bankcols = ncell * CELL
                    cols = slice(c0 * CELL, c0 * CELL + bankcols)
                    nt = len(tiles)
                    psum_agg = psa.tile([128, bankcols], F32)
                    # clear bank (sets has_written)
                    nc.tensor.matmul(
                        psum_agg[:], zeros_t[:, :128], zeros_t[:, :bankcols],
                        start=True, stop=(nt == 0),
                    )
                    if nt:
                        tg0 = tiles[0][0]
                        s_t = sp.tile([128, T_BANK_MAX, CELL], BF16)
                        dap = dstc_t[:, tg0 : tg0 + nt].to_broadcast([128, nt, CELL])
                        iap = bass.AP(
                            tensor=iota_t[:].tensor, offset=iota_t[:].offset,
                            ap=[iota_t[:].ap[0], [0, nt], [1, CELL]],
                        )
                        nc.vector.tensor_tensor(
                            out=s_t[:, :nt, :], in0=dap, in1=iap,
                            op=mybir.AluOpType.is_equal,
                        )
                        msg_t = msgp.tile([128, T_BANK_MAX, D], BF16)
                        for i, (tg, ci) in enumerate(tiles):
                            nc.gpsimd.indirect_dma_start(
                                out=msg_t[:, i, :],
                                out_offset=None,
                                in_=table_d[:],
                                in_offset=bass.IndirectOffsetOnAxis(
                                    ap=sidx_t[:, tg : tg + 1], axis=0
                                ),
                            )
                            nc.tensor.matmul(
                                psum_agg[:, ci * CELL : (ci + 1) * CELL],
                                msg_t[:, i, :],
                                s_t[:, i, :],
                                start=False,
                                stop=(i == nt - 1),
                            )
                    # mean^T = psum * inv_cnt
                    mean_t = mp.tile([128, bankcols], F32R)
                    nc.vector.tensor_tensor(
                        out=mean_t[:], in0=psum_agg[:], in1=inv_t[:, cols],
                        op=mybir.AluOpType.mult,
                    )
                    # own^T: layer1 transposes x shard cells; layer2 loads h1T
                    own_t = ownp.tile([128, bankcols], F32R)
                    if not is_last:
                        for ci in range(ncell):
                            cg = c0 + ci
                            stage = stp.tile([128, CELL], BF16)
                            nc.sync.dma_start(
                                out=stage[:], in_=xsh_d[cg * CELL : (cg + 1) * CELL, :]
                            )
                            tr_p = ptr.tile([128, CELL], BF16)
                            nc.tensor.transpose(
                                out=tr_p[:], in_=stage[:], identity=ident_t[:]
                            )
                            nc.vector.tensor_copy(
                                out=own_t[:, ci * CELL : (ci + 1) * CELL], in_=tr_p[:]
                            )
                    else:
                        nc.sync.dma_start(out=own_t[:], in_=h1T_d[:, cols])
                    # transform: out^T = W_l^T mean^T + W_r^T own^T
                    psum_o = pst.tile([128, bankcols], F32)
                    nc.tensor.matmul(psum_o[:], wl[:], mean_t[:], start=True, stop=False)
                    nc.tensor.matmul(psum_o[:], wr[:], own_t[:], start=False, stop=True)
                    if not is_last:
                        h1t_t = outp.tile([128, bankcols], F32R)
                        nc.scalar.activation(
                            out=h1t_t[:], in_=psum_o[:],
                            func=mybir.ActivationFunctionType.Relu,
                            bias=bb[:], scale=1.0,
                        )
                        nc.sync.dma_start(out=h1T_d[:, cols], in_=h1t_t[:])
                        # row-major bf16 shard for the h1 AllGather
                        for ci in range(ncell):
                            cg = c0 + ci
                            stage2 = stp.tile([128, CELL], BF16)
                            nc.vector.tensor_copy(
                                out=stage2[:], in_=h1t_t[:, ci * CELL : (ci + 1) * CELL]
                            )
                            tr_p2 = ptr.tile([128, CELL], BF16)
                            nc.tensor.transpose(
                                out=tr_p2[:], in_=stage2[:], identity=ident_t[:]
                            )
                            row_t = stp.tile([128, CELL], BF16)
                            nc.vector.tensor_copy(out=row_t[:], in_=tr_p2[:])
                            nc.sync.dma_start(
                                out=h1sh_d[cg * CELL : (cg + 1) * CELL, :], in_=row_t[:]
                            )
                    else:
                        out_t = outp.tile([128, bankcols], BF16)
                        nc.scalar.activation(
                            out=out_t[:], in_=psum_o[:],
                            func=mybir.ActivationFunctionType.Relu,
                            bias=bb[:], scale=1.0,
                        )
                        nc.sync.dma_start(out=out_d[:, cols], in_=out_t[:])

            layer(xfull_d, w_t["wl0"], w_t["wr0"], b_t["b0"], is_last=False)
            nc.gpsimd.collective_compute(
                "AllGather", mybir.AluOpType.bypass, replica_groups=GROUPS,
                ins=[h1sh_d[:]], outs=[h1full_d[:]],
            )
            layer(h1full_d, w_t["wl1"], w_t["wr1"], b_t["b1"], is_last=True)
    nc.finalize()
    return nc


def _schedule(edge_index):
    """Per-core slot schedule shared by both layers (canonical src indices)."""
    src = np.asarray(edge_index[0], dtype=np.int64)
    dst = np.asarray(edge_index[1], dtype=np.int64)
    deg = np.bincount(dst, minlength=N_NODES).astype(np.float32)
    inv_full = 1.0 / np.maximum(deg, 1.0)
    src_canon = ((src // OWN) * N_CANON + (src % OWN)).astype(np.int64)

    cores = []
    cell_counts = np.zeros((N_CORES, N_CELLS), np.int64)
    for k in range(N_CORES):
        m = (dst // OWN) == k
        s_k = src_canon[m]
        dloc = dst[m] - k * OWN
        order = np.argsort(dloc, kind="stable")
        s_k, dloc = s_k[order], dloc[order]
        cell = dloc // CELL
        cell_counts[k] = np.bincount(cell, minlength=N_CELLS)
        cores.append((s_k, dloc, cell))

    T_cells = np.maximum(
        np.ceil(cell_counts.max(axis=0) / 128.0).astype(np.int64), 1
    )
    TOT_T = int(T_cells.sum())
    TOT_S = TOT_T * 128
    tile_base = np.concatenate([[0], np.cumsum(T_cells)])[:-1]  # first tile of cell
    slot_base = tile_base * 128

    sched = []
    for k in range(N_CORES):
        s_k, dloc, cell = cores[k]
        n = len(s_k)
        cnt = cell_counts[k]
        cstart = np.concatenate([[0], np.cumsum(cnt)])[:-1]
        rank = np.arange(n) - cstart[cell]
        slot = slot_base[cell] + rank
        slot_src = np.zeros(TOT_S, np.int64)
        slot_src[slot] = s_k
        dstc_flat = np.full(TOT_S, -1.0, np.float32)
        dstc_flat[slot] = (dloc % CELL).astype(np.float32)
        # slot s -> (t = s//128, p = s%128); device reads sidx/dstc as [p, t]
        sidx_arr = np.ascontiguousarray(slot_src.reshape(TOT_T, 128).T).astype(np.int32)
        dstc_arr = np.ascontiguousarray(
            dstc_flat.reshape(TOT_T, 128).T.astype(ml_dtypes.bfloat16)
        )
        inv_row = np.ones((1, N_CANON), np.float32)
        inv_row[0, :OWN] = inv_full[k * OWN : (k + 1) * OWN]
        sched.append((sidx_arr, dstc_arr, inv_row))
    return sched, T_cells, TOT_T, TOT_S


def _input_maps(sched, x, W_l0, b_l0, W_r0, W_l1, b_l1, W_r1):
    iota = np.arange(CELL).astype(ml_dtypes.bfloat16).reshape(1, CELL)
    x_bf = x.astype(ml_dtypes.bfloat16)
    wl0 = np.ascontiguousarray(W_l0.astype(np.float32))
    wr0 = np.ascontiguousarray(W_r0.astype(np.float32))
    wl1 = np.ascontiguousarray(W_l1.astype(np.float32))
    wr1 = np.ascontiguousarray(W_r1.astype(np.float32))
    b0 = np.ascontiguousarray(b_l0.astype(np.float32).reshape(128, 1))
    b1 = np.ascontiguousarray(b_l1.astype(np.float32).reshape(128, 1))
    in_maps = []
    for k in range(N_CORES):
        sidx_arr, dstc_arr, inv_row = sched[k]
        xsh = np.zeros((N_CANON, D), ml_dtypes.bfloat16)
        xsh[:OWN] = x_bf[k * OWN : (k + 1) * OWN]
        in_maps.append({
            "xsh": xsh,
            "sidx": sidx_arr,
            "dstc": dstc_arr,
            "invc": inv_row,
            "wl0": wl0, "wr0": wr0, "b0": b0,
            "wl1": wl1, "wr1": wr1, "b1": b1,
            "iota": iota,
        })
    return in_maps


def _launch(nc, in_maps):
    import time as _time
    t0 = _time.perf_counter()
    res = run_bass_kernel_spmd(
        nc, [dict(m) for m in in_maps], core_ids=list(range(N_CORES)), trace=False
    )
    wall = int((_time.perf_counter() - t0) * 1e9)
    h = np.empty((N_NODES, D), np.float32)
    for k in range(N_CORES):
        h[k * OWN : (k + 1) * OWN] = (
            res.results[k]["outT"][:, :OWN].astype(np.float32).T
        )
    t = res.exec_time_ns
    return h, (int(t) if t is not None else None), wall


def kernel(x, edge_index, W_l0, b_l0, W_r0, W_l1, b_l1, W_r1):
    x = np.asarray(x, dtype=np.float32)

    sched, T_cells, TOT_T, TOT_S = _schedule(edge_index)
    tkey = ("prog", tuple(T_cells.tolist()))
    if tkey not in _cache:
        _cache[tkey] = _build_program(T_cells)
    nc = _cache[tkey]

    in_maps = _input_maps(sched, x, W_l0, b_l0, W_r0, W_l1, b_l1, W_r1)
    # first launch includes NEFF compile/load; relaunch once for the warm
    # steady-state wall (same convention as the 2-launch baseline, which
    # reported 2*min over its launches).
    h, t1, w1 = _launch(nc, in_maps)
    h2, t2, w2 = _launch(nc, in_maps)
    if t1 is not None and t2 is not None:
        kernel.last_exec_ns = min(t1, t2)
    else:
        # NTFF profiling hook unavailable under this axon client; report the
        # warm launch wall (incl. host<->device transfer) as an upper bound.
        kernel.last_exec_ns = min(w1, w2)
    return h2


if __name__ == "__main__":
    pass
